# revision 78
# baseline (speedup 1.0000x reference)
"""Trainium2 Bass kernel for 16-head causal MultiHeadAttention (bf16).

Problem shapes (hardcoded): x [4, 2048, 1024], Wq/Wk/Wv [1024, 1024],
Wc [1024, 1024], bc [1024].  Output [4, 2048, 1024].

Sharding: 8 cores = (batch b in 0..3) x (head-group g in 0..1).
Each core computes 8 heads (512 of the 1024 hidden dims) for one batch
element, including its partial c_proj contribution.  The host sums the
two partials per batch (fp32) and adds the bias.

All matmul operands are bf16 (same PE rate as fp32r at 512-col moving,
but 2x cheaper DVE/DMA); PSUM accumulation stays fp32.  Verified rel
err vs the fp32 reference ~6e-3.

Per-core schedule (6 "blocks", emitted so the Tile scheduler keeps the
PE dense and the ACT exp stream hidden):
  block n:   QKV projections for token chunk n (12 matmul groups)
             + attention S->exp->mask->O for q-chunk n-1
             + c_proj for token chunks of q-chunks 2(n-4), 2(n-4)+1
The projection/c_proj groups are dispersed between attention k-chunks
as PE filler.  O-matmuls trail the S-matmuls by LAG k-chunks so the PE
never waits on the ACT exp; paired S-matmuls share one 2-bank PSUM
tile so they become ready together and run concurrently on the two
64-row halves of the PE array.  Causal structure: fully-masked 128-col
blocks of diagonal k-chunks are skipped in S, exp and O; the partial
128x128 diagonal block is masked with a tril multiply on DVE.

Softmax denominators ride a ones-column in the V stationary.  Non-hot
segments: reciprocal on DVE, SBUF->DRAM->SBUF broadcast-read bounce,
normalize-multiply deferred into the next segment on gpsimd (zero PE
cost, latency hidden).  The hot final segment instead broadcasts the
reciprocal rows (partitions 0/32) across all 128 partitions with a
selector matmul into PSUM + DVE multiply — a short chain so only it
gates the c_proj tail — while ~24 dummy matmuls keep the PE/HAM warm
through it so the tail c_proj runs at full clock.

Startup: ~30 dummy matmuls on a garbage tile warm the HAM clock gate
through the DMA-bound prologue; weights ship host-pre-arranged in SBUF
layout, with wq 2-chunk pieces interleaved between x0's odd chunks on
the scalar queue (x0 evens on sync) so the first QK groups pipeline
with arrival; wc is deferred to block 2.
"""

import numpy as np
import ml_dtypes

B, T, C = 4, 2048, 1024
H_PER_CORE = 8       # heads per core
HL = 512             # local head width  (8 heads * 64)
D = 64               # head dim
QC = 512             # q-chunk width
NQC = T // QC        # 4
NKC = T // 128       # 16
N_CORES = 8

_CACHE = {}


def _emit(nc, tc, tile, mybir, io):
    import contextlib
    import concourse.bass as bass
    f32, bf16 = mybir.dt.float32, mybir.dt.bfloat16
    Exp = mybir.ActivationFunctionType.Exp
    xT, wq, wk, wv, wc, tril, out = (
        io["xT"], io["wq"], io["wk"], io["wv"], io["wc"],
        io["tril"], io["out"],
    )

    from contextlib import ExitStack

    with ExitStack() as ctx:
        persist = ctx.enter_context(tc.tile_pool(name="persist", bufs=1))
        # Q^T / K^T / O^T: [512 dims, 2048 toks] as [128, 4 chunks, 2048]
        qt = persist.tile([128, 4, T], bf16)
        kt = persist.tile([128, 4, T], bf16)
        ot = persist.tile([128, 4, T], bf16)
        # V': [tok-part, k-chunk, head, dim+ones]
        vp = persist.tile([128, NKC, H_PER_CORE, D + 1], bf16)
        wq_sb = persist.tile([128, 8, HL], bf16, tag="wqs")
        wk_sb = persist.tile([128, 8, HL], bf16, tag="wks")
        wv_sb = persist.tile([128, 8, HL], bf16, tag="wvs")
        wc_sb = persist.tile([128, 4, C], bf16, tag="wcs")
        tril_sb = persist.tile([128, 2, 128], bf16, tag="tril")
        sel_sb = persist.tile([33, 128], f32, tag="sel")
        # never written: operand for warm-up dummy matmuls (content is
        # irrelevant; results are discarded)
        garb = persist.tile([128, 2, 128], bf16, tag="garb")

        # prologue DMAs: weights arrive host-pre-arranged in the SBUF
        # layout (contiguous per-partition lines, fast descriptors).
        # Arrival order is tuned so the first QK groups can pipeline:
        # wq 2-chunk pieces interleave with x0's odd chunks on the scalar
        # queue (x0 evens ride sync), wk follows, wv on gpsimd; wc is
        # deferred to block 2 (first needed by c_proj in block 4).
        nc.vector.memset(vp[:, :, :, D], 1.0)

        pA = ctx.enter_context(tc.tile_pool(name="pA", bufs=2, space="PSUM"))
        pwp = ctx.enter_context(tc.tile_pool(name="pw", bufs=2, space="PSUM"))
        pop = ctx.enter_context(tc.tile_pool(name="po", bufs=2, space="PSUM"))
        xtp = ctx.enter_context(tc.tile_pool(name="xtp", bufs=16))
        ewp = ctx.enter_context(tc.tile_pool(name="ewp", bufs=6))
        dbp = ctx.enter_context(tc.tile_pool(name="dbp", bufs=2))
        stp = ctx.enter_context(tc.tile_pool(name="stp", bufs=4))
        drp = ctx.enter_context(tc.tile_pool(name="drp", bufs=4, space="DRAM"))

        x_tiles = {}

        def emit_x_dma(n, interleave_w=False):
            """Load x^T [1024, tok chunk n] as 8 single-DMA tiles (bf16).
            n=0: evens on sync, odds on scalar interleaved with the wq
            2-chunk pieces; then wk/wv follow."""
            ts = []
            for kc in range(8):
                if interleave_w and kc % 2 == 1:
                    c2 = kc // 2 * 2
                    nc.scalar.dma_start(out=wq_sb[:, c2:c2 + 2, :],
                                        in_=wq[:, c2:c2 + 2, :])
                xt = xtp.tile([128, QC], bf16, tag="xt")
                eng = nc.scalar if (n <= 1 and kc % 2 == 1) else nc.sync
                eng.dma_start(
                    out=xt[:],
                    in_=xT[kc * 128:(kc + 1) * 128, n * QC:(n + 1) * QC])
                ts.append(xt)
            if interleave_w:
                for c2 in range(0, 8, 2):
                    nc.scalar.dma_start(out=wk_sb[:, c2:c2 + 2, :],
                                        in_=wk[:, c2:c2 + 2, :])
                for h2 in range(2):
                    nc.gpsimd.dma_start(
                        out=wv_sb[:, 4 * h2:4 * h2 + 4, :],
                        in_=wv[:, 4 * h2:4 * h2 + 4, :])
            x_tiles[n] = ts

        def xs(n, kc):
            return x_tiles[n][kc][:]

        def qk_group(n, mc, wsb, dst, ev):
            p = pA.tile([128, QC], f32, tag="pA")
            for kc in range(8):
                nc.tensor.matmul(
                    out=p[:], lhsT=wsb[:, kc, mc * 128:(mc + 1) * 128],
                    rhs=xs(n, kc), start=(kc == 0), stop=(kc == 7))
            dst_ap = dst[:, mc, n * QC:(n + 1) * QC]
            if ev == "act":
                nc.scalar.copy(dst_ap, p[:])
            else:
                nc.vector.tensor_copy(dst_ap, p[:])

        def v_group(n, mt, ev):
            p = pA.tile([128, QC], f32, tag="pA")
            for kc in range(8):
                nc.tensor.matmul(
                    out=p[:], lhsT=xs(n, kc)[:, mt * 128:(mt + 1) * 128],
                    rhs=wv_sb[:, kc, :], start=(kc == 0), stop=(kc == 7))
            gm = n * 4 + mt
            out_ap = vp[:, gm, :, 0:D]
            in_ap = p.rearrange("p (h d) -> p h d", d=D)
            if ev == "act":
                nc.scalar.copy(out_ap, in_ap)
            else:
                nc.vector.tensor_copy(out_ap, in_ap)

        def p3_group(qc, j, n2, ev):
            mt = 4 * qc + j
            p = pA.tile([128, QC], f32, tag="pA")
            for kd in range(4):
                nc.tensor.matmul(
                    out=p[:],
                    lhsT=ot[:, kd, mt * 128:(mt + 1) * 128],
                    rhs=wc_sb[:, kd, n2 * QC:(n2 + 1) * QC],
                    start=(kd == 0), stop=(kd == 3))
            st = stp.tile([128, QC], bf16, tag="st")
            if ev == "act":
                nc.scalar.copy(st[:], p[:])
            else:
                nc.vector.tensor_copy(st[:], p[:])
            (nc.scalar if ev == "act" else nc.sync).dma_start(
                out=out[mt * 128:(mt + 1) * 128, n2 * QC:(n2 + 1) * QC],
                in_=st[:])

        # deferred normalize (sel-matmul + mul) shared across so_blocks:
        # flushed one k-chunk into the next segment so the in-order PE
        # queue never stalls on the reciprocal chain
        pending_norm = [None]

        def so_block(qc):
            """Generator: S->exp->mask->O for q-chunk qc, all 4 head pairs.

            O-matmuls trail the S-matmuls by LAG k-chunks so the PE never
            waits on the ACT exp.  Yields once per k-chunk (the filler
            insertion points)."""
            K = 4 * qc + 4
            LAG = 4 if K > 4 else 3
            for hp in range(4):
                po_a = pop.tile([128, QC], f32, tag="po")
                po_b = pop.tile([128, QC], f32, tag="po")
                hot = (qc == 3 and hp == 3)
                if hot:
                    # denominator tile for the selector-matmul broadcast:
                    # memset early (no po dependency) so the final chain
                    # is shorter by one DVE op
                    d33 = dbp.tile([33, QC], f32, tag="d33")
                    nc.vector.memset(d33[:], 1.0)
                ews = {}

                def o_pair(kc, qc=qc, hp=hp, po_a=po_a, po_b=po_b, ews=ews):
                    off = (kc - 4 * qc) * 128 if kc >= 4 * qc else 0
                    ew = ews.pop(kc)
                    for hi, (h, po_t) in ((0, (2 * hp, po_a)),
                                          (1, (2 * hp + 1, po_b))):
                        nc.tensor.matmul(
                            out=po_t[0:D + 1, off:QC],
                            lhsT=vp[:, kc, h, :],
                            rhs=ew[:, hi, off:QC],
                            start=(kc == 0), stop=(kc == K - 1),
                            skip_group_check=True)

                flush_kc = 3 if K <= 8 else 5
                for kc in range(K):
                    if kc == flush_kc and pending_norm[0] is not None:
                        # previous segment's deferred broadcast matmul +
                        # normalize: by now its reciprocal is long done, so
                        # the PE does not stall on it
                        pending_norm[0]()
                        pending_norm[0] = None
                    off = (kc - 4 * qc) * 128 if kc >= 4 * qc else 0
                    pw_t = pwp.tile([128, 2, QC], f32, tag="pw")
                    # paired S-matmuls: one PSUM tile, adjacent emission ->
                    # both become ready together and run on the two 64-row
                    # halves of the PE array concurrently
                    for hi, r0 in ((0, 0), (1, 64)):
                        nc.tensor.matmul(
                            out=pw_t[:, hi, off:QC],
                            lhsT=kt[r0:r0 + 64, hp,
                                    kc * 128:(kc + 1) * 128],
                            rhs=qt[r0:r0 + 64, hp,
                                   qc * QC + off:(qc + 1) * QC],
                            start=True, stop=True, tile_position=(r0, 0))
                    ew = ewp.tile([128, 2, QC], bf16, tag="ew")
                    ews[kc] = ew
                    nc.scalar.activation(
                        ew[:, :, off:QC], pw_t[:, :, off:QC], Exp,
                        scale=0.125)
                    # causal mask on the partial diagonal 128-col block
                    if kc >= 4 * qc:
                        nc.vector.tensor_mul(
                            ew[:, :, off:off + 128],
                            ew[:, :, off:off + 128],
                            tril_sb[:, :, :])
                    if kc >= LAG:
                        o_pair(kc - LAG)
                    yield
                for kc in range(max(0, K - LAG), K):
                    o_pair(kc)
                # normalization.  Non-hot segments: reciprocal on DVE, then
                # an SBUF->DRAM->SBUF broadcast-read bounce (zero PE cost,
                # latency hidden by the one-segment deferral of the gpsimd
                # multiply).  Hot (final) segment: selector-matmul
                # broadcast + DVE multiply — a short chain so only it
                # gates the c_proj tail — with dummy matmuls bridging the
                # PE so HAM stays warm for that tail.
                qsl = slice(qc * QC, (qc + 1) * QC)
                if hot:
                    warm = pA.tile([128, QC], f32, tag="pA")
                    for _ in range(24):
                        nc.tensor.matmul(out=warm[:, 0:256],
                                         lhsT=garb[:, 0, :],
                                         rhs=garb[:, :, :],
                                         start=True, stop=True)
                    with tc.high_priority():
                        nc.vector.tensor_copy(d33[0:1, :], po_a[D:D + 1, :])
                        nc.vector.tensor_copy(d33[32:33, :],
                                              po_b[D:D + 1, :])
                        nc.vector.reciprocal_approx_fast(d33[:], d33[:])
                        nc.vector.tensor_copy(ot[0:64, hp, qsl],
                                              po_a[0:D, :])
                        nc.vector.tensor_copy(ot[64:128, hp, qsl],
                                              po_b[0:D, :])
                        pbk = pA.tile([128, QC], f32, tag="pA")
                        nc.tensor.matmul(out=pbk[:], lhsT=sel_sb[:],
                                         rhs=d33[:], start=True, stop=True)
                        nc.vector.tensor_mul(ot[:, hp, qsl],
                                             ot[:, hp, qsl], pbk[:])
                else:
                    d_sb = dbp.tile([1, 2, QC], f32, tag="dsb")
                    nc.vector.tensor_copy(d_sb[0:1, 0, :], po_a[D:D + 1, :])
                    nc.vector.tensor_copy(d_sb[0:1, 1, :], po_b[D:D + 1, :])
                    nc.vector.reciprocal_approx_fast(d_sb[:], d_sb[:])
                    dr = drp.tile([2, QC], f32, tag="dr")
                    nc.sync.dma_start(out=dr[:], in_=d_sb[:])
                    db = dbp.tile([128, QC], bf16, tag="db")
                    d0 = dr[:]
                    nc.gpsimd.dma_start(
                        out=db[:],
                        in_=bass.AP(tensor=d0.tensor, offset=d0.offset,
                                    ap=[[QC, 2], [0, 64], [1, QC]]))
                    nc.vector.tensor_copy(ot[0:64, hp, qsl], po_a[0:D, :])
                    nc.vector.tensor_copy(ot[64:128, hp, qsl], po_b[0:D, :])
                    pending_norm[0] = (
                        lambda hp=hp, qsl=qsl, db=db: nc.gpsimd.tensor_mul(
                            ot[:, hp, qsl], ot[:, hp, qsl], db[:]))

        def block_fillers(n):
            fs = []
            ev = "dve"
            if n == 2:
                # wc load deferred out of the startup bandwidth window
                fs.append(lambda: nc.gpsimd.dma_start(out=wc_sb, in_=wc))
            if n + 1 < NQC:
                fs.append(lambda n=n: emit_x_dma(n + 1))
            if n == 0:
                # chunk-interleaved Q/K accumulation: two concurrent
                # groups double the PE work per arriving x/w chunk, so
                # HAM stays warm through the DMA-trickle phase
                def qk2(mc):
                    pq = pA.tile([128, QC], f32, tag="pA")
                    pk = pA.tile([128, QC], f32, tag="pA")
                    for kc in range(8):
                        nc.tensor.matmul(
                            out=pq[:],
                            lhsT=wq_sb[:, kc, mc * 128:(mc + 1) * 128],
                            rhs=xs(0, kc), start=(kc == 0), stop=(kc == 7),
                            skip_group_check=True)
                        nc.tensor.matmul(
                            out=pk[:],
                            lhsT=wk_sb[:, kc, mc * 128:(mc + 1) * 128],
                            rhs=xs(0, kc), start=(kc == 0), stop=(kc == 7),
                            skip_group_check=True)
                    nc.vector.tensor_copy(qt[:, mc, 0:QC], pq[:])
                    nc.vector.tensor_copy(kt[:, mc, 0:QC], pk[:])
                for mc in range(4):
                    fs.append(lambda mc=mc: qk2(mc))
            elif n < NQC:
                for mc in range(4):
                    fs.append(
                        lambda n=n, mc=mc: qk_group(n, mc, wq_sb, qt, ev))
                    fs.append(
                        lambda n=n, mc=mc: qk_group(n, mc, wk_sb, kt, ev))
            if n < NQC:
                for mt in range(4):
                    fs.append(lambda n=n, mt=mt: v_group(n, mt, ev))
            if n >= 4:
                qcs = (0, 1, 2) if n == 4 else (3,)
                for qc in qcs:
                    for j in range(4):
                        for n2 in range(2):
                            pev = "dve" if (n == 4 or (j + n2) % 2) else "act"
                            fs.append(lambda qc=qc, j=j, n2=n2, pev=pev:
                                      p3_group(qc, j, n2, pev))
            return fs

        nc.sync.dma_start(out=tril_sb[:, 0, :], in_=tril)
        nc.sync.dma_start(out=tril_sb[:, 1, :], in_=tril)
        nc.sync.dma_start(out=sel_sb[:], in_=io["sel"])
        emit_x_dma(0, interleave_w=True)
        # warm-up: keep the PE busy through the DMA-bound prologue so HAM
        # reaches K=8/8 before the first real matmul group.  The operand
        # memset has no input deps, so these start right after the preamble.
        nc.vector.memset(garb[:], 0.0)
        warm0 = pA.tile([128, QC], f32, tag="pA")
        for _ in range(30):
            nc.tensor.matmul(out=warm0[:, 0:256], lhsT=garb[:, 0, :],
                             rhs=garb[:, :, :], start=True, stop=True)
        for n in range(6):
            fillers = block_fillers(n)
            if n == 0 or n == 5:
                for f in fillers:
                    f()
                continue
            qc = n - 1
            n_bi = 4 * (4 * qc + 4)
            rate = len(fillers) / n_bi
            acc = 0.0
            for _ in so_block(qc):
                acc += rate
                while acc >= 1.0 and fillers:
                    fillers.pop(0)()
                    acc -= 1.0
            for f in fillers:
                f()


def build_program():
    """Build and compile the per-core Bass program (cached)."""
    if "nc" in _CACHE:
        return _CACHE["nc"]
    import concourse.bacc as bacc
    import concourse.tile as tile
    from concourse import mybir

    bf16 = mybir.dt.bfloat16
    nc = bacc.Bacc("TRN2", target_bir_lowering=False, debug=False,
                   num_devices=N_CORES)
    io = {
        "xT": nc.dram_tensor("xT", [C, T], bf16, kind="ExternalInput").ap(),
        # weights ship host-pre-arranged in SBUF layout: [part, chunk, cols]
        "wq": nc.dram_tensor("wq", [128, 8, HL], bf16,
                             kind="ExternalInput").ap(),
        "wk": nc.dram_tensor("wk", [128, 8, HL], bf16,
                             kind="ExternalInput").ap(),
        "wv": nc.dram_tensor("wv", [128, 8, HL], bf16,
                             kind="ExternalInput").ap(),
        "wc": nc.dram_tensor("wc", [128, 4, C], bf16,
                             kind="ExternalInput").ap(),
        "tril": nc.dram_tensor("tril", [128, 128], bf16,
                               kind="ExternalInput").ap(),
        "sel": nc.dram_tensor("sel", [33, 128], mybir.dt.float32,
                              kind="ExternalInput").ap(),
        "out": nc.dram_tensor("out", [T, C], bf16, kind="ExternalOutput").ap(),
    }
    with tile.TileContext(nc) as tc:
        _emit(nc, tc, tile, mybir, io)
    nc.compile()
    _CACHE["nc"] = nc
    return nc


def make_in_maps(x, Wq, Wk, Wv, Wc):
    bf16 = ml_dtypes.bfloat16
    x = np.asarray(x, dtype=np.float32)
    Wq = np.asarray(Wq, dtype=np.float32).astype(bf16)
    Wk = np.asarray(Wk, dtype=np.float32).astype(bf16)
    Wv = np.asarray(Wv, dtype=np.float32).astype(bf16)
    Wc = np.asarray(Wc, dtype=np.float32).astype(bf16)

    i_idx = np.arange(128)[:, None]
    j_idx = np.arange(128)[None, :]
    tril = (j_idx >= i_idx).astype(bf16)

    sel = np.zeros((33, 128), dtype=np.float32)
    sel[0, 0:64] = 1.0
    sel[32, 64:128] = 1.0

    def warr(w):
        # [1024, HL] -> SBUF layout [128 part, 8 chunk, HL]
        return np.ascontiguousarray(
            w.reshape(8, 128, HL).transpose(1, 0, 2))

    in_maps = []
    for b in range(B):
        xT = np.ascontiguousarray(x[b].T).astype(bf16)
        for g in range(2):
            sl = slice(g * HL, (g + 1) * HL)
            wcg = Wc[sl, :]  # [512, 1024]
            in_maps.append({
                "xT": xT,
                "wq": warr(Wq[:, sl]),
                "wk": warr(Wk[:, sl]),
                "wv": warr(Wv[:, sl]),
                "wc": np.ascontiguousarray(
                    wcg.reshape(4, 128, C).transpose(1, 0, 2)),
                "tril": tril,
                "sel": sel,
            })
    return in_maps


def kernel(x, Wq, Wk, Wv, Wc, bc):
    from concourse.bass_utils import run_bass_kernel_spmd

    nc = build_program()
    in_maps = make_in_maps(x, Wq, Wk, Wv, Wc)
    res = run_bass_kernel_spmd(nc, in_maps, core_ids=list(range(N_CORES)))
    bc = np.asarray(bc, dtype=np.float32)
    out = np.empty((B, T, C), dtype=np.float32)
    for b in range(B):
        out[b] = (res.results[2 * b]["out"].astype(np.float32)
                  + res.results[2 * b + 1]["out"].astype(np.float32) + bc)
    return out

